# revision 24
# baseline (speedup 1.0000x reference)
"""LocalRNN Trainium2 kernel: GLU -> pointwise conv -> 9-step windowed LSTM.

Full inputs in, full output out. Sharding: batch across 8 cores (2 batches/core).

v5 design notes:
- W_hh matmuls for steps 1..FP8_LAST run in fp8-e4m3 DoubleRow perf mode
  (2 contraction tiles per matmul, 0.5 cycles/row): per bank 2 fp8 matmuls
  instead of 4 f32r ones. The final step(s) run exact f32r to keep the
  output error ~1.7e-2 (measured on the reference instance; gate is 2e-2).
- h for fp8 steps is stored quantized (x4) in e4m3 pair-layout tiles
  hTp[phase][p2] = [128, 2, NT]: partition p, slot i holds h-dim
  256*p2 + 128*i + p, which DoubleRow contracts as out += lhsT[:,i,:].T @
  rhs[:,i,:] -- no cross-partition shuffles needed.
- W8 = e4m3(W_hh.T * 16); everything that lands in PSUM is uniformly scaled
  x64 (G table, biases, and the f32r W_hh are pre-scaled x64 on the host;
  fp8 product = 16W * 4h = 64 W.h), and the ACT ops undo it for free with
  their input scale= operand.
- Per-step input gates G enter PSUM via identity-matmul injection (PE).
- Cell work split across engines: DVE t1/c-update, GpSimd t2/h-mul
  (GPSIMD cannot touch PSUM), ACT the activations; the tanh(c)/h-mul pair
  of each unit is emitted one unit later (software pipelining) so no engine
  blocks in-order on a mid-unit dependency.
- DMA order: bias, x(batch0), wf, whh8, x(batch1); the 4MB f32r W_hh is only
  needed by the late f32r steps and loads in the background.
- Output returned in transposed layout; host does the final transpose.
"""
from contextlib import ExitStack

import ml_dtypes
import numpy as np

import concourse.bass as bass
import concourse.mybir as mybir
import concourse.tile as tile
from concourse import bacc, bass_utils
from concourse.masks import make_identity

F32 = mybir.dt.float32
F32R = mybir.dt.float32r
FP8 = mybir.dt.float8e4
AF = mybir.ActivationFunctionType
DR = mybir.MatmulPerfMode.DoubleRow

N_CORES = 8
B_PER_CORE = 2          # batches per core
L = 512                 # sequence length
NT = B_PER_CORE * L     # tokens per core = 1024
D = 512                 # model dim
DH = 256                # GLU half dim
G4 = 4 * D              # 2048 gate rows
K = 9                   # window size
PAD = K - 1             # 8
LW = PAD + L            # 520: per-batch padded G row width

FP8_LAST = 7            # last step whose W_hh matmul runs in fp8
W_SCALE = 64.0          # fp8 weight quantization scale
H_SCALE = 1.0           # h stored unscaled (plain-mul store, no stt needed)
G_SCALE = W_SCALE * H_SCALE   # uniform PSUM scale (64)
INV_G = 1.0 / G_SCALE

_cache = {}


def _build():
    nc = bacc.Bacc(
        trn_type="TRN2", target_bir_lowering=False, debug=False, num_devices=N_CORES
    )

    x_d = nc.dram_tensor("x", [NT, D], F32, kind="ExternalInput").ap()
    wf_d = nc.dram_tensor("wf", [DH, G4], F32, kind="ExternalInput").ap()     # (w_ih@conv_w).T perm, x64
    whh_d = nc.dram_tensor("whh", [D, G4], F32, kind="ExternalInput").ap()    # w_hh.T perm, x64
    w8_d = nc.dram_tensor("w8", [2 * 128, 2 * G4], FP8, kind="ExternalInput").ap()  # pair-layout, x16
    bias_d = nc.dram_tensor("bias", [128, 32], F32, kind="ExternalInput").ap()  # x64
    out_d = nc.dram_tensor("out", [D, NT], F32, kind="ExternalOutput").ap()   # transposed out

    with tile.TileContext(nc) as tc, ExitStack() as top:
        const_pool = top.enter_context(tc.tile_pool(name="const", bufs=1))
        w_pool = top.enter_context(tc.tile_pool(name="weights", bufs=1))
        state_pool = top.enter_context(tc.tile_pool(name="state", bufs=1))

        ident_f32 = const_pool.tile([128, 128], F32, tag="idf")
        make_identity(nc, ident_f32[:])
        ident = const_pool.tile([128, 128], F32R, tag="idr")
        nc.scalar.copy(ident[:], ident_f32[:])
        zeros8 = const_pool.tile([128, PAD], F32, tag="z8")
        nc.gpsimd.memset(zeros8[:], 0.0)
        bias_sb = const_pool.tile([128, 32], F32, tag="bias")

        whh = [w_pool.tile([128, G4], F32R, tag=f"whh{dk}", name=f"whh{dk}")
               for dk in range(4)]
        w8p = [w_pool.tile([128, 2, G4], FP8, tag=f"w8_{p2}", name=f"w8_{p2}")
               for p2 in range(2)]

        # G table, quadrant-major: gt4[j][:, q, b*LW + t] = gate chunk (4q+j)
        gt4 = [state_pool.tile([128, 4, B_PER_CORE * LW], F32R, tag=f"gt{j}",
                               name=f"gt{j}")
               for j in range(4)]
        # fp8 h for DoubleRow steps: [p, i, t] = h[256*p2 + 128*i + p, t] * 4
        hTp = [[state_pool.tile([128, 2, NT], FP8, tag=f"h8_{p}_{p2}",
                                name=f"h8_{p}_{p2}")
                for p2 in range(2)] for p in range(2)]
        cT = [state_pool.tile([128, NT], F32, tag=f"c{j}", name=f"c{j}") for j in range(4)]

        tp = top.enter_context(tc.tile_pool(name="tmp", bufs=2))
        late = []  # pools/tiles allocated after prep frees its SBUF space

        def h_dst(k, b, j):
            """where cell (k,b,j) writes h, and in which format."""
            if k < FP8_LAST:       # feeds an fp8 DoubleRow step
                return hTp[k % 2][j // 2][:, j % 2, b * 512:(b + 1) * 512], True
            hT7 = late[1]          # feeds an f32r step (or is the output)
            return hT7[k % 2][j][:, b * 512:(b + 1) * 512], False

        def cell0(j, b):
            """step 0: c = sig(I)*tanh(G); h = sig(O)*tanh(c)."""
            cs = cT[j][:, b * 512:(b + 1) * 512]
            g0 = b * LW  # step-0 slice offset (pad col 0..7 + G cols 0..503)
            tI = tp.tile([128, 512], F32, tag="t1", name="tI0")
            nc.scalar.activation(tI[:], gt4[j][:, 0, g0:g0 + 512], AF.Sigmoid,
                                 scale=INV_G)
            tG = tp.tile([128, 512], F32, tag="tG", name="tG0")
            nc.scalar.activation(tG[:], gt4[j][:, 3, g0:g0 + 512], AF.Tanh,
                                 scale=INV_G)
            tO = tp.tile([128, 512], F32, tag="tSig", name="tO0")
            nc.scalar.activation(tO[:], gt4[j][:, 2, g0:g0 + 512], AF.Sigmoid,
                                 scale=INV_G)
            nc.gpsimd.tensor_mul(cs, tI[:], tG[:])
            tTc = tp.tile([128, 512], F32, tag="tTc", name="tTc0")
            nc.scalar.activation(tTc[:], cs, AF.Tanh)
            hs, _ = h_dst(0, b, j)
            nc.gpsimd.tensor_mul(hs, tO[:], tTc[:])

        def cell_a(j, b, P, k, st):
            """activations + c update from psum P [128, 4, 512] = I|F|O|G."""
            cs = cT[j][:, b * 512:(b + 1) * 512]
            tSig = tp.tile([128, 3, 512], F32, tag="tSig", name="tSig")
            nc.scalar.activation(tSig[:], P[:, 0:3, :], AF.Sigmoid, scale=INV_G)
            tG = tp.tile([128, 512], F32, tag="tG", name="tG")
            nc.scalar.activation(tG[:], P[:, 3, :], AF.Tanh, scale=INV_G)
            t1 = tp.tile([128, 512], F32, tag="t1", name="t1")
            t2 = late[0].tile([128, 512], F32, tag="t2", name="t2")
            nc.gpsimd.tensor_mul(t2[:], tSig[:, 1, :], cs)
            nc.vector.tensor_mul(t1[:], tSig[:, 0, :], tG[:])
            nc.vector.tensor_add(cs, t1[:], t2[:])
            st["tSig"] = tSig

        def cell_b(j, b, k, st):
            """tanh(c), h-mul, and the final-step store."""
            cs = cT[j][:, b * 512:(b + 1) * 512]
            tSig = st["tSig"]
            tTc = tp.tile([128, 512], F32, tag="tTc", name="tTc")
            nc.scalar.activation(tTc[:], cs, AF.Tanh)
            hs, _ = h_dst(k, b, j)
            nc.gpsimd.tensor_mul(hs, tSig[:, 2, :], tTc[:])
            if k == K - 1:
                nc.sync.dma_start(
                    out_d[j * 128:(j + 1) * 128, b * 512:(b + 1) * 512].bitcast(F32R),
                    hs,
                )

        # one uniform PSUM pool for the whole kernel: 2 slots x 4 banks
        psg = top.enter_context(tc.tile_pool(name="psg", bufs=2, space="PSUM"))

        # ---------------- prep: GLU -> u -> G table; step 0 interleaved ----------------
        with ExitStack() as prep:
            utp = prep.enter_context(tc.tile_pool(name="utp", bufs=1))
            wfp = prep.enter_context(tc.tile_pool(name="wfp", bufs=1))

            wf = [wfp.tile([128, G4], F32R, tag=f"wf{ck}", name=f"wf{ck}")
                  for ck in range(2)]

            nc.sync.dma_start(bias_sb[:], bias_d)
            uT = [utp.tile([128, NT], F32R, tag=f"uT{ci}", name=f"uT{ci}")
                  for ci in range(2)]
            with ExitStack() as glu:
                xp = glu.enter_context(tc.tile_pool(name="xp2", bufs=2))
                xab = glu.enter_context(tc.tile_pool(name="xab", bufs=1))
                for half in range(2):
                    xa = [xab.tile([128, 512], F32, tag=f"xa{ci}", name=f"xa{ci}")
                          for ci in range(2)]
                    xb = [xab.tile([128, 512], F32, tag=f"xb{ci}", name=f"xb{ci}")
                          for ci in range(2)]
                    for tl in range(4):
                        ti = half * 4 + tl
                        xt = xp.tile([128, D], F32, tag="x", name="xt")
                        nc.sync.dma_start(xt[:], x_d[ti * 128:(ti + 1) * 128, :])
                        if half == 0 and tl == 3:
                            # x tiles 0..3 issued; fp8 weights are tiny and
                            # needed first (step 1), queue them next
                            for p2 in range(2):
                                nc.sync.dma_start(
                                    w8p[p2][:],
                                    w8_d[p2 * 128:(p2 + 1) * 128, :],
                                )
                        ptp = psg.tile([128, 4, 512], F32, tag="P", name="Ptr")
                        for ci in range(4):
                            ptr = ptp[:, ci, 0:128]
                            nc.tensor.transpose(
                                ptr, xt[:, ci * 128:(ci + 1) * 128], ident_f32[:]
                            )
                            dst = xa[ci] if ci < 2 else xb[ci - 2]
                            nc.vector.tensor_copy(dst[:, tl * 128:(tl + 1) * 128], ptr)
                    if half == 1:
                        # after all x tiles: wf (needed by g_phase), then the
                        # f32r weights (needed only by the late exact steps)
                        for ck in range(2):
                            for hh in range(2):
                                nc.sync.dma_start(
                                    wf[ck][:, hh * 1024:(hh + 1) * 1024],
                                    wf_d[ck * 128:(ck + 1) * 128,
                                         hh * 1024:(hh + 1) * 1024].bitcast(F32R),
                                )
                        for dk in range(4):
                            for hh in range(2):
                                nc.sync.dma_start(
                                    whh[dk][:, hh * 1024:(hh + 1) * 1024],
                                    whh_d[dk * 128:(dk + 1) * 128,
                                          hh * 1024:(hh + 1) * 1024].bitcast(F32R),
                                )
                    for ci in range(2):
                        sgt = tp.tile([128, 3, 512], F32, tag="tSig", name="sgt")
                        nc.scalar.activation(sgt[:, 0, :], xb[ci][:], AF.Sigmoid)
                        nc.gpsimd.tensor_mul(
                            uT[ci][:, half * 512:(half + 1) * 512], xa[ci][:],
                            sgt[:, 0, :]
                        )

            def g_phase(b):
                # pad columns get the x64 (b_ih+b_hh) bias (u=0 there)
                for j in range(4):
                    for q in range(4):
                        i = 4 * q + j
                        nc.scalar.activation(
                            gt4[j][:, q, b * LW:b * LW + PAD], zeros8[:],
                            AF.Identity, bias=bias_sb[:, 16 + i:16 + i + 1],
                        )
                for j in range(4):
                    P = psg.tile([128, 4, 512], F32, tag="P", name="Pg")
                    for q in range(4):
                        for ck in range(2):
                            nc.tensor.matmul(
                                P[:, q, :],
                                wf[ck][:, (4 * q + j) * 128:(4 * q + j + 1) * 128],
                                uT[ck][:, b * 512:(b + 1) * 512],
                                start=(ck == 0), stop=(ck == 1),
                            )
                    for q in range(4):
                        nc.vector.tensor_scalar_add(
                            gt4[j][:, q, b * LW + PAD:b * LW + LW],
                            P[:, q, :],
                            bias_sb[:, 4 * q + j:4 * q + j + 1],
                        )

            g_phase(0)
            for j in range(4):
                cell0(j, 0)
            g_phase(1)
            for j in range(4):
                cell0(j, 1)

        # ---------------- LSTM steps 1..8 ----------------
        late.append(top.enter_context(tc.tile_pool(name="tmp2", bufs=2)))
        l_pool = top.enter_context(tc.tile_pool(name="late_h", bufs=1))
        late.append([[l_pool.tile([128, NT], F32R, tag=f"h7_{p}_{j}",
                                  name=f"h7_{p}_{j}")
                      for j in range(4)] for p in range(2)])

        pend = [None]

        def unit(k, b, j):
            P = psg.tile([128, 4, 512], F32, tag="P", name="P")
            g0 = b * LW + k
            for q in range(4):
                # banks 0..2 get G via PE identity injection; bank 3 gets it
                # via a DVE add below (cheaper than a 512-row injection now
                # that the W_hh matmuls are fp8)
                if q < 3:
                    nc.tensor.matmul(
                        P[:, q, :], ident[:], gt4[j][:, q, g0:g0 + 512],
                        start=True, stop=False,
                    )
                if k <= FP8_LAST:
                    for p2 in range(2):
                        nc.tensor.matmul(
                            P[:, q, :],
                            w8p[p2][:, :, (4 * q + j) * 128:(4 * q + j + 1) * 128],
                            hTp[(k + 1) % 2][p2][:, :, b * 512:(b + 1) * 512],
                            start=(q == 3 and p2 == 0), stop=(p2 == 1),
                            perf_mode=DR,
                        )
                else:
                    hT7 = late[1]
                    for dk in range(4):
                        nc.tensor.matmul(
                            P[:, q, :],
                            whh[dk][:, (4 * q + j) * 128:(4 * q + j + 1) * 128],
                            hT7[(k + 1) % 2][dk][:, b * 512:(b + 1) * 512],
                            start=(q == 3 and dk == 0), stop=(dk == 3),
                        )
            nc.vector.tensor_add(
                P[:, 3, :], P[:, 3, :], gt4[j][:, 3, g0:g0 + 512]
            )
            st = {}
            cell_a(j, b, P, k, st)
            if pend[0] is not None:
                cell_b(*pend[0])
            pend[0] = (j, b, k, st)

        for k in range(1, K):
            for b in range(B_PER_CORE):
                for j in range(4):
                    unit(k, b, j)
        cell_b(*pend[0])

    nc.compile()
    return nc


def _make_in_maps(inputs):
    x = np.asarray(inputs["x"], dtype=np.float32)
    conv_w = np.asarray(inputs["conv_w"], dtype=np.float64)
    conv_b = np.asarray(inputs["conv_b"], dtype=np.float64)
    w_ih = np.asarray(inputs["w_ih"], dtype=np.float64)
    w_hh = np.asarray(inputs["w_hh"], dtype=np.float32)
    b_ih = np.asarray(inputs["b_ih"], dtype=np.float64)
    b_hh = np.asarray(inputs["b_hh"], dtype=np.float64)

    # gate permutation: torch order i,f,g,o -> i,f,o,g
    perm = np.concatenate([
        np.arange(0, D), np.arange(D, 2 * D),
        np.arange(3 * D, 4 * D), np.arange(2 * D, 3 * D),
    ])
    wf = (w_ih @ conv_w)[perm]                                  # [2048, 256]
    bias_mm = (b_ih + b_hh + w_ih @ conv_b)[perm]               # real columns
    bias_pad = (b_ih + b_hh)[perm]                              # zero-padded columns
    whh_p = w_hh[perm]                                          # [2048, 512]

    bias_both = np.concatenate([
        bias_mm.astype(np.float32).reshape(16, 128).T,
        bias_pad.astype(np.float32).reshape(16, 128).T,
    ], axis=1) * np.float32(G_SCALE)                            # [128, 32]

    whh_t = np.ascontiguousarray(whh_p.T)                       # [512, 2048]
    # fp8 pair layout: w8[p2*128 + p, i*2048 + g] = e4m3(16 * W[g, 256p2+128i+p])
    w8 = (whh_t * np.float32(W_SCALE)).reshape(2, 2, 128, G4)   # [p2, i, p, g]
    w8 = np.ascontiguousarray(
        w8.transpose(0, 2, 1, 3).reshape(2 * 128, 2 * G4)
    ).astype(ml_dtypes.float8_e4m3fn)

    shared = {
        "wf": np.ascontiguousarray((wf.T * G_SCALE).astype(np.float32)),  # [256, 2048]
        "whh": np.ascontiguousarray(whh_t * np.float32(G_SCALE)),         # [512, 2048]
        "w8": w8,
        "bias": np.ascontiguousarray(bias_both),
    }
    in_maps = []
    for c in range(N_CORES):
        m = dict(shared)
        m["x"] = np.ascontiguousarray(
            x[c * B_PER_CORE:(c + 1) * B_PER_CORE].reshape(NT, D)
        )
        in_maps.append(m)
    return in_maps


def kernel(x, conv_w, conv_b, w_ih, w_hh, b_ih, b_hh):
    if "nc" not in _cache:
        _cache["nc"] = _build()
    nc = _cache["nc"]

    in_maps = _make_in_maps(dict(
        x=x, conv_w=conv_w, conv_b=conv_b, w_ih=w_ih, w_hh=w_hh,
        b_ih=b_ih, b_hh=b_hh,
    ))

    res = bass_utils.run_bass_kernel_spmd(nc, in_maps, core_ids=list(range(N_CORES)))
    out = np.concatenate(
        [np.ascontiguousarray(r["out"].T).reshape(B_PER_CORE, L, D)
         for r in res.results], axis=0
    )
    return out


# revision 27
# speedup vs baseline: 1.1694x; 1.1694x over previous
"""LocalRNN Trainium2 kernel: GLU -> pointwise conv -> 9-step windowed LSTM.

Full inputs in, full output out. Sharding: batch across 8 cores (2 batches/core).

v5 design notes:
- W_hh matmuls for steps 1..FP8_LAST run in fp8-e4m3 DoubleRow perf mode
  (2 contraction tiles per matmul, 0.5 cycles/row): per bank 2 fp8 matmuls
  instead of 4 f32r ones. The final step(s) run exact f32r to keep the
  output error ~1.7e-2 (measured on the reference instance; gate is 2e-2).
- h for fp8 steps is stored quantized (x4) in e4m3 pair-layout tiles
  hTp[phase][p2] = [128, 2, NT]: partition p, slot i holds h-dim
  256*p2 + 128*i + p, which DoubleRow contracts as out += lhsT[:,i,:].T @
  rhs[:,i,:] -- no cross-partition shuffles needed.
- W8 = e4m3(W_hh.T * 16); everything that lands in PSUM is uniformly scaled
  x64 (G table, biases, and the f32r W_hh are pre-scaled x64 on the host;
  fp8 product = 16W * 4h = 64 W.h), and the ACT ops undo it for free with
  their input scale= operand.
- Per-step input gates G enter PSUM via identity-matmul injection (PE).
- Cell work split across engines: DVE t1/c-update, GpSimd t2/h-mul
  (GPSIMD cannot touch PSUM), ACT the activations; the tanh(c)/h-mul pair
  of each unit is emitted one unit later (software pipelining) so no engine
  blocks in-order on a mid-unit dependency.
- DMA order: bias, x(batch0), wf, whh8, x(batch1); the 4MB f32r W_hh is only
  needed by the late f32r steps and loads in the background.
- Output returned in transposed layout; host does the final transpose.
"""
from contextlib import ExitStack

import ml_dtypes
import numpy as np

import concourse.bass as bass
import concourse.mybir as mybir
import concourse.tile as tile
from concourse import bacc, bass_utils
from concourse.masks import make_identity

F32 = mybir.dt.float32
F32R = mybir.dt.float32r
FP8 = mybir.dt.float8e4
AF = mybir.ActivationFunctionType
DR = mybir.MatmulPerfMode.DoubleRow

N_CORES = 8
B_PER_CORE = 2          # batches per core
L = 512                 # sequence length
NT = B_PER_CORE * L     # tokens per core = 1024
D = 512                 # model dim
DH = 256                # GLU half dim
G4 = 4 * D              # 2048 gate rows
K = 9                   # window size
PAD = K - 1             # 8
LW = PAD + L            # 520: per-batch padded G row width

FP8_LAST = 7            # last step whose W_hh matmul runs in fp8
W_SCALE = 64.0          # fp8 weight quantization scale
H_SCALE = 1.0           # h stored unscaled (plain-mul store, no stt needed)
G_SCALE = W_SCALE * H_SCALE   # uniform PSUM scale (64)
INV_G = 1.0 / G_SCALE

_cache = {}


def _build():
    nc = bacc.Bacc(
        trn_type="TRN2", target_bir_lowering=False, debug=False, num_devices=N_CORES
    )

    x_d = nc.dram_tensor("x", [NT, D], F32, kind="ExternalInput").ap()
    wf_d = nc.dram_tensor("wf", [DH, G4], F32, kind="ExternalInput").ap()     # (w_ih@conv_w).T perm, x64
    whh_d = nc.dram_tensor("whh", [D, G4], F32, kind="ExternalInput").ap()    # w_hh.T perm, x64
    w8_d = nc.dram_tensor("w8", [2 * 128, 2 * G4], FP8, kind="ExternalInput").ap()  # pair-layout, x16
    bias_d = nc.dram_tensor("bias", [128, 32], F32, kind="ExternalInput").ap()  # x64
    out_d = nc.dram_tensor("out", [D, NT], F32, kind="ExternalOutput").ap()   # transposed out

    with tile.TileContext(nc) as tc, ExitStack() as top:
        const_pool = top.enter_context(tc.tile_pool(name="const", bufs=1))
        w_pool = top.enter_context(tc.tile_pool(name="weights", bufs=1))
        state_pool = top.enter_context(tc.tile_pool(name="state", bufs=1))

        ident_f32 = const_pool.tile([128, 128], F32, tag="idf")
        make_identity(nc, ident_f32[:])
        ident = const_pool.tile([128, 128], F32R, tag="idr")
        nc.scalar.copy(ident[:], ident_f32[:])
        zeros8 = const_pool.tile([128, PAD], F32, tag="z8")
        nc.gpsimd.memset(zeros8[:], 0.0)
        bias_sb = const_pool.tile([128, 32], F32, tag="bias")

        whh = [w_pool.tile([128, G4], F32R, tag=f"whh{dk}", name=f"whh{dk}")
               for dk in range(4)]
        w8p = [w_pool.tile([128, 2, G4], FP8, tag=f"w8_{p2}", name=f"w8_{p2}")
               for p2 in range(2)]

        # G table, quadrant-major: gt4[j][:, q, b*LW + t] = gate chunk (4q+j)
        gt4 = [state_pool.tile([128, 4, B_PER_CORE * LW], F32R, tag=f"gt{j}",
                               name=f"gt{j}")
               for j in range(4)]
        # fp8 h for DoubleRow steps: [p, i, t] = h[256*p2 + 128*i + p, t] * 4
        hTp = [[state_pool.tile([128, 2, NT], FP8, tag=f"h8_{p}_{p2}",
                                name=f"h8_{p}_{p2}")
                for p2 in range(2)] for p in range(2)]
        cT = [state_pool.tile([128, NT], F32, tag=f"c{j}", name=f"c{j}") for j in range(4)]

        tp = top.enter_context(tc.tile_pool(name="tmp", bufs=2))
        late = []  # pools/tiles allocated after prep frees its SBUF space

        def h_dst(k, b, j):
            """where cell (k,b,j) writes h, and in which format."""
            if k < FP8_LAST:       # feeds an fp8 DoubleRow step
                return hTp[k % 2][j // 2][:, j % 2, b * 512:(b + 1) * 512], True
            hT7 = late[1]          # feeds an f32r step (or is the output)
            return hT7[k % 2][j][:, b * 512:(b + 1) * 512], False

        def cell0(j, b):
            """step 0: c = sig(I)*tanh(G); h = sig(O)*tanh(c)."""
            cs = cT[j][:, b * 512:(b + 1) * 512]
            g0 = b * LW  # step-0 slice offset (pad col 0..7 + G cols 0..503)
            tI = tp.tile([128, 512], F32, tag="t1", name="tI0")
            nc.scalar.activation(tI[:], gt4[j][:, 0, g0:g0 + 512], AF.Sigmoid,
                                 scale=INV_G)
            tG = tp.tile([128, 512], F32, tag="tG", name="tG0")
            nc.scalar.activation(tG[:], gt4[j][:, 3, g0:g0 + 512], AF.Tanh,
                                 scale=INV_G)
            tO = tp.tile([128, 512], F32, tag="tSig", name="tO0")
            nc.scalar.activation(tO[:], gt4[j][:, 2, g0:g0 + 512], AF.Sigmoid,
                                 scale=INV_G)
            nc.gpsimd.tensor_mul(cs, tI[:], tG[:])
            tTc = tp.tile([128, 512], F32, tag="tTc", name="tTc0")
            nc.scalar.activation(tTc[:], cs, AF.Tanh)
            hs, _ = h_dst(0, b, j)
            nc.vector.tensor_mul(hs, tO[:], tTc[:])

        def cell_a(j, b, P, k, st):
            """activations + c update from psum P [128, 4, 512] = I|F|O|G."""
            cs = cT[j][:, b * 512:(b + 1) * 512]
            tSig = tp.tile([128, 3, 512], F32, tag="tSig", name="tSig")
            nc.scalar.activation(tSig[:], P[:, 0:3, :], AF.Sigmoid, scale=INV_G)
            tG = tp.tile([128, 512], F32, tag="tG", name="tG")
            nc.scalar.activation(tG[:], P[:, 3, :], AF.Tanh, scale=INV_G)
            t1 = tp.tile([128, 512], F32, tag="t1", name="t1")
            t2 = late[0].tile([128, 512], F32, tag="t2", name="t2")
            nc.gpsimd.tensor_mul(t2[:], tSig[:, 1, :], cs)
            nc.vector.tensor_mul(t1[:], tSig[:, 0, :], tG[:])
            nc.vector.tensor_add(cs, t1[:], t2[:])
            st["tSig"] = tSig

        def cell_b(j, b, k, st):
            """tanh(c), h-mul, and the final-step store."""
            cs = cT[j][:, b * 512:(b + 1) * 512]
            tSig = st["tSig"]
            tTc = tp.tile([128, 512], F32, tag="tTc", name="tTc")
            nc.scalar.activation(tTc[:], cs, AF.Tanh)
            hs, _ = h_dst(k, b, j)
            nc.vector.tensor_mul(hs, tSig[:, 2, :], tTc[:])
            if k == K - 1:
                nc.sync.dma_start(
                    out_d[j * 128:(j + 1) * 128, b * 512:(b + 1) * 512].bitcast(F32R),
                    hs,
                )

        # one uniform PSUM pool for the whole kernel: 2 slots x 4 banks
        psg = top.enter_context(tc.tile_pool(name="psg", bufs=2, space="PSUM"))

        # ---------------- prep: GLU -> u -> G table; step 0 interleaved ----------------
        with ExitStack() as prep:
            utp = prep.enter_context(tc.tile_pool(name="utp", bufs=1))
            wfp = prep.enter_context(tc.tile_pool(name="wfp", bufs=1))

            wf = [wfp.tile([128, G4], F32R, tag=f"wf{ck}", name=f"wf{ck}")
                  for ck in range(2)]

            nc.sync.dma_start(bias_sb[:], bias_d)
            uT = [utp.tile([128, NT], F32R, tag=f"uT{ci}", name=f"uT{ci}")
                  for ci in range(2)]
            with ExitStack() as glu:
                xp = glu.enter_context(tc.tile_pool(name="xp2", bufs=2))
                xab = glu.enter_context(tc.tile_pool(name="xab", bufs=1))
                for half in range(2):
                    xa = [xab.tile([128, 512], F32, tag=f"xa{ci}", name=f"xa{ci}")
                          for ci in range(2)]
                    xb = [xab.tile([128, 512], F32, tag=f"xb{ci}", name=f"xb{ci}")
                          for ci in range(2)]
                    for tl in range(4):
                        ti = half * 4 + tl
                        xt = xp.tile([128, D], F32, tag="x", name="xt")
                        nc.sync.dma_start(xt[:], x_d[ti * 128:(ti + 1) * 128, :])
                        if half == 0 and tl == 3:
                            # x tiles 0..3 issued; fp8 weights are tiny and
                            # needed first (step 1), queue them next
                            for p2 in range(2):
                                nc.sync.dma_start(
                                    w8p[p2][:],
                                    w8_d[p2 * 128:(p2 + 1) * 128, :],
                                )
                        ptp = psg.tile([128, 4, 512], F32, tag="P", name="Ptr")
                        for ci in range(4):
                            ptr = ptp[:, ci, 0:128]
                            nc.tensor.transpose(
                                ptr, xt[:, ci * 128:(ci + 1) * 128], ident_f32[:]
                            )
                            dst = xa[ci] if ci < 2 else xb[ci - 2]
                            nc.vector.tensor_copy(dst[:, tl * 128:(tl + 1) * 128], ptr)
                    if half == 1:
                        # after all x tiles: wf (needed by g_phase), then the
                        # f32r weights (needed only by the late exact steps)
                        for ck in range(2):
                            for hh in range(2):
                                nc.sync.dma_start(
                                    wf[ck][:, hh * 1024:(hh + 1) * 1024],
                                    wf_d[ck * 128:(ck + 1) * 128,
                                         hh * 1024:(hh + 1) * 1024].bitcast(F32R),
                                )
                        for dk in range(4):
                            for hh in range(2):
                                nc.sync.dma_start(
                                    whh[dk][:, hh * 1024:(hh + 1) * 1024],
                                    whh_d[dk * 128:(dk + 1) * 128,
                                          hh * 1024:(hh + 1) * 1024].bitcast(F32R),
                                )
                    for ci in range(2):
                        sgt = tp.tile([128, 3, 512], F32, tag="tSig", name="sgt")
                        nc.scalar.activation(sgt[:, 0, :], xb[ci][:], AF.Sigmoid)
                        nc.gpsimd.tensor_mul(
                            uT[ci][:, half * 512:(half + 1) * 512], xa[ci][:],
                            sgt[:, 0, :]
                        )

            def g_phase(b):
                # pad columns get the x64 (b_ih+b_hh) bias (u=0 there)
                for j in range(4):
                    for q in range(4):
                        i = 4 * q + j
                        nc.scalar.activation(
                            gt4[j][:, q, b * LW:b * LW + PAD], zeros8[:],
                            AF.Identity, bias=bias_sb[:, 16 + i:16 + i + 1],
                        )
                for j in range(4):
                    P = psg.tile([128, 4, 512], F32, tag="P", name="Pg")
                    for q in range(4):
                        for ck in range(2):
                            nc.tensor.matmul(
                                P[:, q, :],
                                wf[ck][:, (4 * q + j) * 128:(4 * q + j + 1) * 128],
                                uT[ck][:, b * 512:(b + 1) * 512],
                                start=(ck == 0), stop=(ck == 1),
                            )
                    for q in range(4):
                        nc.vector.tensor_scalar_add(
                            gt4[j][:, q, b * LW + PAD:b * LW + LW],
                            P[:, q, :],
                            bias_sb[:, 4 * q + j:4 * q + j + 1],
                        )

            g_phase(0)
            for j in range(4):
                cell0(j, 0)
            g_phase(1)
            for j in range(4):
                cell0(j, 1)

        # ---------------- LSTM steps 1..8 ----------------
        late.append(top.enter_context(tc.tile_pool(name="tmp2", bufs=2)))
        l_pool = top.enter_context(tc.tile_pool(name="late_h", bufs=1))
        late.append([[l_pool.tile([128, NT], F32R, tag=f"h7_{p}_{j}",
                                  name=f"h7_{p}_{j}")
                      for j in range(4)] for p in range(2)])

        pend = [None]

        def unit(k, b, j):
            P = psg.tile([128, 4, 512], F32, tag="P", name="P")
            g0 = b * LW + k
            for q in range(4):
                nc.tensor.matmul(
                    P[:, q, :], ident[:], gt4[j][:, q, g0:g0 + 512],
                    start=True, stop=False,
                )
                if k <= FP8_LAST:
                    for p2 in range(2):
                        nc.tensor.matmul(
                            P[:, q, :],
                            w8p[p2][:, :, (4 * q + j) * 128:(4 * q + j + 1) * 128],
                            hTp[(k + 1) % 2][p2][:, :, b * 512:(b + 1) * 512],
                            start=False, stop=(p2 == 1), perf_mode=DR,
                        )
                else:
                    hT7 = late[1]
                    for dk in range(4):
                        nc.tensor.matmul(
                            P[:, q, :],
                            whh[dk][:, (4 * q + j) * 128:(4 * q + j + 1) * 128],
                            hT7[(k + 1) % 2][dk][:, b * 512:(b + 1) * 512],
                            start=False, stop=(dk == 3),
                        )
            st = {}
            cell_a(j, b, P, k, st)
            if pend[0] is not None:
                cell_b(*pend[0])
            pend[0] = (j, b, k, st)

        for k in range(1, K):
            for b in range(B_PER_CORE):
                for j in range(4):
                    unit(k, b, j)
        cell_b(*pend[0])

    nc.compile()
    return nc


def _make_in_maps(inputs):
    x = np.asarray(inputs["x"], dtype=np.float32)
    conv_w = np.asarray(inputs["conv_w"], dtype=np.float64)
    conv_b = np.asarray(inputs["conv_b"], dtype=np.float64)
    w_ih = np.asarray(inputs["w_ih"], dtype=np.float64)
    w_hh = np.asarray(inputs["w_hh"], dtype=np.float32)
    b_ih = np.asarray(inputs["b_ih"], dtype=np.float64)
    b_hh = np.asarray(inputs["b_hh"], dtype=np.float64)

    # gate permutation: torch order i,f,g,o -> i,f,o,g
    perm = np.concatenate([
        np.arange(0, D), np.arange(D, 2 * D),
        np.arange(3 * D, 4 * D), np.arange(2 * D, 3 * D),
    ])
    wf = (w_ih @ conv_w)[perm]                                  # [2048, 256]
    bias_mm = (b_ih + b_hh + w_ih @ conv_b)[perm]               # real columns
    bias_pad = (b_ih + b_hh)[perm]                              # zero-padded columns
    whh_p = w_hh[perm]                                          # [2048, 512]

    bias_both = np.concatenate([
        bias_mm.astype(np.float32).reshape(16, 128).T,
        bias_pad.astype(np.float32).reshape(16, 128).T,
    ], axis=1) * np.float32(G_SCALE)                            # [128, 32]

    whh_t = np.ascontiguousarray(whh_p.T)                       # [512, 2048]
    # fp8 pair layout: w8[p2*128 + p, i*2048 + g] = e4m3(16 * W[g, 256p2+128i+p])
    w8 = (whh_t * np.float32(W_SCALE)).reshape(2, 2, 128, G4)   # [p2, i, p, g]
    w8 = np.ascontiguousarray(
        w8.transpose(0, 2, 1, 3).reshape(2 * 128, 2 * G4)
    ).astype(ml_dtypes.float8_e4m3fn)

    shared = {
        "wf": np.ascontiguousarray((wf.T * G_SCALE).astype(np.float32)),  # [256, 2048]
        "whh": np.ascontiguousarray(whh_t * np.float32(G_SCALE)),         # [512, 2048]
        "w8": w8,
        "bias": np.ascontiguousarray(bias_both),
    }
    in_maps = []
    for c in range(N_CORES):
        m = dict(shared)
        m["x"] = np.ascontiguousarray(
            x[c * B_PER_CORE:(c + 1) * B_PER_CORE].reshape(NT, D)
        )
        in_maps.append(m)
    return in_maps


def kernel(x, conv_w, conv_b, w_ih, w_hh, b_ih, b_hh):
    if "nc" not in _cache:
        _cache["nc"] = _build()
    nc = _cache["nc"]

    in_maps = _make_in_maps(dict(
        x=x, conv_w=conv_w, conv_b=conv_b, w_ih=w_ih, w_hh=w_hh,
        b_ih=b_ih, b_hh=b_hh,
    ))

    res = bass_utils.run_bass_kernel_spmd(nc, in_maps, core_ids=list(range(N_CORES)))
    out = np.concatenate(
        [np.ascontiguousarray(r["out"].T).reshape(B_PER_CORE, L, D)
         for r in res.results], axis=0
    )
    return out


# revision 32
# speedup vs baseline: 1.1855x; 1.0138x over previous
"""LocalRNN Trainium2 kernel: GLU -> pointwise conv -> 9-step windowed LSTM.

Full inputs in, full output out. Sharding: batch across 8 cores (2 batches/core).

v5 design notes:
- W_hh matmuls for steps 1..FP8_LAST run in fp8-e4m3 DoubleRow perf mode
  (2 contraction tiles per matmul, 0.5 cycles/row): per bank 2 fp8 matmuls
  instead of 4 f32r ones. The final step(s) run exact f32r to keep the
  output error ~1.7e-2 (measured on the reference instance; gate is 2e-2).
- h for fp8 steps is stored quantized (x4) in e4m3 pair-layout tiles
  hTp[phase][p2] = [128, 2, NT]: partition p, slot i holds h-dim
  256*p2 + 128*i + p, which DoubleRow contracts as out += lhsT[:,i,:].T @
  rhs[:,i,:] -- no cross-partition shuffles needed.
- W8 = e4m3(W_hh.T * 16); everything that lands in PSUM is uniformly scaled
  x64 (G table, biases, and the f32r W_hh are pre-scaled x64 on the host;
  fp8 product = 16W * 4h = 64 W.h), and the ACT ops undo it for free with
  their input scale= operand.
- Per-step input gates G enter PSUM via identity-matmul injection (PE).
- Cell work split across engines: DVE t1/c-update, GpSimd t2/h-mul
  (GPSIMD cannot touch PSUM), ACT the activations; the tanh(c)/h-mul pair
  of each unit is emitted one unit later (software pipelining) so no engine
  blocks in-order on a mid-unit dependency.
- DMA order: bias, x(batch0), wf, whh8, x(batch1); the 4MB f32r W_hh is only
  needed by the late f32r steps and loads in the background.
- Output returned in transposed layout; host does the final transpose.
"""
from contextlib import ExitStack

import ml_dtypes
import numpy as np

import concourse.bass as bass
import concourse.mybir as mybir
import concourse.tile as tile
from concourse import bacc, bass_utils
from concourse.masks import make_identity

F32 = mybir.dt.float32
F32R = mybir.dt.float32r
FP8 = mybir.dt.float8e4
AF = mybir.ActivationFunctionType
DR = mybir.MatmulPerfMode.DoubleRow

N_CORES = 8
B_PER_CORE = 2          # batches per core
L = 512                 # sequence length
NT = B_PER_CORE * L     # tokens per core = 1024
D = 512                 # model dim
DH = 256                # GLU half dim
G4 = 4 * D              # 2048 gate rows
K = 9                   # window size
PAD = K - 1             # 8
LW = PAD + L            # 520: per-batch padded G row width

FP8_LAST = 7            # last step whose W_hh matmul runs in fp8
W_SCALE = 64.0          # fp8 weight quantization scale
H_SCALE = 1.0           # h stored unscaled (plain-mul store, no stt needed)
G_SCALE = W_SCALE * H_SCALE   # uniform PSUM scale (64)
INV_G = 1.0 / G_SCALE

_cache = {}


def _build():
    nc = bacc.Bacc(
        trn_type="TRN2", target_bir_lowering=False, debug=False, num_devices=N_CORES
    )

    x_d = nc.dram_tensor("x", [NT, D], F32, kind="ExternalInput").ap()
    wf_d = nc.dram_tensor("wf", [DH, G4], F32, kind="ExternalInput").ap()     # (w_ih@conv_w).T perm, x64
    whh_d = nc.dram_tensor("whh", [D, G4], F32, kind="ExternalInput").ap()    # w_hh.T perm, x64
    w8_d = nc.dram_tensor("w8", [2 * 128, 2 * G4], FP8, kind="ExternalInput").ap()  # pair-layout, x16
    bias_d = nc.dram_tensor("bias", [128, 32], F32, kind="ExternalInput").ap()  # x64
    out_d = nc.dram_tensor("out", [D, NT], F32, kind="ExternalOutput").ap()   # transposed out

    with tile.TileContext(nc) as tc, ExitStack() as top:
        const_pool = top.enter_context(tc.tile_pool(name="const", bufs=1))
        w_pool = top.enter_context(tc.tile_pool(name="weights", bufs=1))
        state_pool = top.enter_context(tc.tile_pool(name="state", bufs=1))

        ident_f32 = const_pool.tile([128, 128], F32, tag="idf")
        make_identity(nc, ident_f32[:])
        ident = const_pool.tile([128, 128], F32R, tag="idr")
        nc.scalar.copy(ident[:], ident_f32[:])
        zeros8 = const_pool.tile([128, PAD], F32, tag="z8")
        nc.gpsimd.memset(zeros8[:], 0.0)
        bias_sb = const_pool.tile([128, 32], F32, tag="bias")

        whh = [w_pool.tile([128, G4], F32R, tag=f"whh{dk}", name=f"whh{dk}")
               for dk in range(4)]
        w8p = [w_pool.tile([128, 2, G4], FP8, tag=f"w8_{p2}", name=f"w8_{p2}")
               for p2 in range(2)]

        # G table, quadrant-major: gt4[j][:, q, b*LW + t] = gate chunk (4q+j)
        gt4 = [state_pool.tile([128, 4, B_PER_CORE * LW], F32R, tag=f"gt{j}",
                               name=f"gt{j}")
               for j in range(4)]
        # fp8 h for DoubleRow steps: [p, i, t] = h[256*p2 + 128*i + p, t] * 4
        hTp = [[state_pool.tile([128, 2, NT], FP8, tag=f"h8_{p}_{p2}",
                                name=f"h8_{p}_{p2}")
                for p2 in range(2)] for p in range(2)]
        cT = [state_pool.tile([128, NT], F32, tag=f"c{j}", name=f"c{j}") for j in range(4)]

        tp = top.enter_context(tc.tile_pool(name="tmp", bufs=2))
        late = []  # pools/tiles allocated after prep frees its SBUF space

        def h_dst(k, b, j):
            """where cell (k,b,j) writes h, and in which format."""
            if k < FP8_LAST:       # feeds an fp8 DoubleRow step
                return hTp[k % 2][j // 2][:, j % 2, b * 512:(b + 1) * 512], True
            hT7 = late[1]          # feeds an f32r step (or is the output)
            return hT7[k % 2][j][:, b * 512:(b + 1) * 512], False

        def cell0(j, b):
            """step 0: c = sig(I)*tanh(G); h = sig(O)*tanh(c)."""
            cs = cT[j][:, b * 512:(b + 1) * 512]
            g0 = b * LW  # step-0 slice offset (pad col 0..7 + G cols 0..503)
            tI = tp.tile([128, 512], F32, tag="t1", name="tI0")
            nc.scalar.activation(tI[:], gt4[j][:, 0, g0:g0 + 512], AF.Sigmoid,
                                 scale=INV_G)
            tG = tp.tile([128, 512], F32, tag="tG", name="tG0")
            nc.scalar.activation(tG[:], gt4[j][:, 3, g0:g0 + 512], AF.Tanh,
                                 scale=INV_G)
            tO = tp.tile([128, 512], F32, tag="tSig", name="tO0", bufs=3)
            nc.scalar.activation(tO[:], gt4[j][:, 2, g0:g0 + 512], AF.Sigmoid,
                                 scale=INV_G)
            nc.gpsimd.tensor_mul(cs, tI[:], tG[:])
            tTc = tp.tile([128, 512], F32, tag="tTc", name="tTc0")
            nc.scalar.activation(tTc[:], cs, AF.Tanh)
            hs, _ = h_dst(0, b, j)
            nc.vector.tensor_mul(hs, tO[:], tTc[:])

        def cell_a(j, b, P, k, st):
            """activations + c update from psum P [128, 4, 512] = I|F|O|G."""
            cs = cT[j][:, b * 512:(b + 1) * 512]
            tSig = tp.tile([128, 3, 512], F32, tag="tSig", name="tSig", bufs=3)
            nc.scalar.activation(tSig[:], P[:, 0:3, :], AF.Sigmoid, scale=INV_G)
            tG = tp.tile([128, 512], F32, tag="tG", name="tG")
            nc.scalar.activation(tG[:], P[:, 3, :], AF.Tanh, scale=INV_G)
            t1 = tp.tile([128, 512], F32, tag="t1", name="t1")
            t2 = late[0].tile([128, 512], F32, tag="t2", name="t2")
            nc.gpsimd.tensor_mul(t2[:], tSig[:, 1, :], cs)
            nc.vector.tensor_mul(t1[:], tSig[:, 0, :], tG[:])
            nc.vector.tensor_add(cs, t1[:], t2[:])
            st["tSig"] = tSig

        def cell_b(j, b, k, st):
            """tanh(c), h-mul, and the final-step store."""
            cs = cT[j][:, b * 512:(b + 1) * 512]
            tSig = st["tSig"]
            tTc = tp.tile([128, 512], F32, tag="tTc", name="tTc")
            nc.scalar.activation(tTc[:], cs, AF.Tanh)
            hs, _ = h_dst(k, b, j)
            nc.vector.tensor_mul(hs, tSig[:, 2, :], tTc[:])
            if k == K - 1:
                nc.sync.dma_start(
                    out_d[j * 128:(j + 1) * 128, b * 512:(b + 1) * 512].bitcast(F32R),
                    hs,
                )

        # one uniform PSUM pool for the whole kernel: 2 slots x 4 banks
        psg = top.enter_context(tc.tile_pool(name="psg", bufs=2, space="PSUM"))

        # ---------------- prep: GLU -> u -> G table; step 0 interleaved ----------------
        with ExitStack() as prep:
            utp = prep.enter_context(tc.tile_pool(name="utp", bufs=1))
            wfp = prep.enter_context(tc.tile_pool(name="wfp", bufs=1))

            wf = [wfp.tile([128, G4], F32R, tag=f"wf{ck}", name=f"wf{ck}")
                  for ck in range(2)]

            nc.sync.dma_start(bias_sb[:], bias_d)
            uT = [utp.tile([128, NT], F32R, tag=f"uT{ci}", name=f"uT{ci}")
                  for ci in range(2)]
            with ExitStack() as glu:
                xp = glu.enter_context(tc.tile_pool(name="xp2", bufs=2))
                # GLU in token-major right off the DMA, then transpose the
                # 256-wide u (half the transposes/copies of transposing x)
                for ti in range(8):
                    xt = xp.tile([128, D], F32, tag="x", name="xt")
                    for hh in range(2):
                        nc.sync.dma_start(
                            xt[:, hh * 256:(hh + 1) * 256],
                            x_d[ti * 128:(ti + 1) * 128, hh * 256:(hh + 1) * 256],
                        )
                    if ti == 3:
                        # x tiles 0..3 issued; fp8 weights are tiny and
                        # needed first (step 1), then the rest of x, then
                        # the f32r weights (needed only by the late steps)
                        for p2 in range(2):
                            nc.sync.dma_start(
                                w8p[p2][:], w8_d[p2 * 128:(p2 + 1) * 128, :],
                            )
                    sgt = xp.tile([128, 256], F32, tag="sg", name="sgt")
                    nc.scalar.activation(sgt[:], xt[:, 256:512], AF.Sigmoid)
                    if ti == 3:
                        # wf via the ACT hwdge queue; lands before g_phase(0)
                        for ck in range(2):
                            for hh in range(2):
                                nc.scalar.dma_start(
                                    wf[ck][:, hh * 1024:(hh + 1) * 1024],
                                    wf_d[ck * 128:(ck + 1) * 128,
                                         hh * 1024:(hh + 1) * 1024].bitcast(F32R),
                                )
                    ut = xp.tile([128, 256], F32, tag="ut", name="ut")
                    nc.gpsimd.tensor_mul(ut[:], xt[:, 0:256], sgt[:])
                    ptp = psg.tile([128, 4, 512], F32, tag="P", name="Ptr")
                    for ci in range(2):
                        ptr = ptp[:, ci, 0:128]
                        nc.tensor.transpose(
                            ptr, ut[:, ci * 128:(ci + 1) * 128], ident_f32[:]
                        )
                        nc.vector.tensor_copy(
                            uT[ci][:, ti * 128:(ti + 1) * 128], ptr
                        )
                for dk in range(4):
                    for hh in range(2):
                        nc.sync.dma_start(
                            whh[dk][:, hh * 1024:(hh + 1) * 1024],
                            whh_d[dk * 128:(dk + 1) * 128,
                                  hh * 1024:(hh + 1) * 1024].bitcast(F32R),
                        )

            def g_phase(b):
                # pad columns get the x64 (b_ih+b_hh) bias (u=0 there)
                for j in range(4):
                    for q in range(4):
                        i = 4 * q + j
                        nc.scalar.activation(
                            gt4[j][:, q, b * LW:b * LW + PAD], zeros8[:],
                            AF.Identity, bias=bias_sb[:, 16 + i:16 + i + 1],
                        )
                for j in range(4):
                    P = psg.tile([128, 4, 512], F32, tag="P", name="Pg")
                    for q in range(4):
                        for ck in range(2):
                            nc.tensor.matmul(
                                P[:, q, :],
                                wf[ck][:, (4 * q + j) * 128:(4 * q + j + 1) * 128],
                                uT[ck][:, b * 512:(b + 1) * 512],
                                start=(ck == 0), stop=(ck == 1),
                            )
                    for q in range(4):
                        nc.vector.tensor_scalar_add(
                            gt4[j][:, q, b * LW + PAD:b * LW + LW],
                            P[:, q, :],
                            bias_sb[:, 4 * q + j:4 * q + j + 1],
                        )

            g_phase(0)
            for j in range(4):
                cell0(j, 0)
            g_phase(1)

        # ---------------- LSTM steps 1..8 ----------------
        late.append(top.enter_context(tc.tile_pool(name="tmp2", bufs=2)))
        l_pool = top.enter_context(tc.tile_pool(name="late_h", bufs=1))
        late.append([[l_pool.tile([128, NT], F32R, tag=f"h7_{p}_{j}",
                                  name=f"h7_{p}_{j}")
                      for j in range(4)] for p in range(2)])

        pend = [None]

        def unit(k, b, j):
            P = psg.tile([128, 4, 512], F32, tag="P", name="P")
            g0 = b * LW + k
            for q in range(4):
                nc.tensor.matmul(
                    P[:, q, :], ident[:], gt4[j][:, q, g0:g0 + 512],
                    start=True, stop=False,
                )
                if k <= FP8_LAST:
                    for p2 in range(2):
                        nc.tensor.matmul(
                            P[:, q, :],
                            w8p[p2][:, :, (4 * q + j) * 128:(4 * q + j + 1) * 128],
                            hTp[(k + 1) % 2][p2][:, :, b * 512:(b + 1) * 512],
                            start=False, stop=(p2 == 1), perf_mode=DR,
                        )
                else:
                    hT7 = late[1]
                    for dk in range(4):
                        nc.tensor.matmul(
                            P[:, q, :],
                            whh[dk][:, (4 * q + j) * 128:(4 * q + j + 1) * 128],
                            hT7[(k + 1) % 2][dk][:, b * 512:(b + 1) * 512],
                            start=False, stop=(dk == 3),
                        )
            st = {}
            cell_a(j, b, P, k, st)
            if pend[0] is not None:
                cell_b(*pend[0])
            pend[0] = (j, b, k, st)

        for k in range(1, K):
            for b in range(B_PER_CORE):
                for j in range(4):
                    unit(k, b, j)
                    if k == 1 and b == 0:
                        # batch 1's step-0 cells overlap batch 0's first units
                        cell0(j, 1)
        cell_b(*pend[0])

    nc.compile()
    return nc


def _make_in_maps(inputs):
    x = np.asarray(inputs["x"], dtype=np.float32)
    conv_w = np.asarray(inputs["conv_w"], dtype=np.float64)
    conv_b = np.asarray(inputs["conv_b"], dtype=np.float64)
    w_ih = np.asarray(inputs["w_ih"], dtype=np.float64)
    w_hh = np.asarray(inputs["w_hh"], dtype=np.float32)
    b_ih = np.asarray(inputs["b_ih"], dtype=np.float64)
    b_hh = np.asarray(inputs["b_hh"], dtype=np.float64)

    # gate permutation: torch order i,f,g,o -> i,f,o,g
    perm = np.concatenate([
        np.arange(0, D), np.arange(D, 2 * D),
        np.arange(3 * D, 4 * D), np.arange(2 * D, 3 * D),
    ])
    wf = (w_ih @ conv_w)[perm]                                  # [2048, 256]
    bias_mm = (b_ih + b_hh + w_ih @ conv_b)[perm]               # real columns
    bias_pad = (b_ih + b_hh)[perm]                              # zero-padded columns
    whh_p = w_hh[perm]                                          # [2048, 512]

    bias_both = np.concatenate([
        bias_mm.astype(np.float32).reshape(16, 128).T,
        bias_pad.astype(np.float32).reshape(16, 128).T,
    ], axis=1) * np.float32(G_SCALE)                            # [128, 32]

    whh_t = np.ascontiguousarray(whh_p.T)                       # [512, 2048]
    # fp8 pair layout: w8[p2*128 + p, i*2048 + g] = e4m3(16 * W[g, 256p2+128i+p])
    w8 = (whh_t * np.float32(W_SCALE)).reshape(2, 2, 128, G4)   # [p2, i, p, g]
    w8 = np.ascontiguousarray(
        w8.transpose(0, 2, 1, 3).reshape(2 * 128, 2 * G4)
    ).astype(ml_dtypes.float8_e4m3fn)

    shared = {
        "wf": np.ascontiguousarray((wf.T * G_SCALE).astype(np.float32)),  # [256, 2048]
        "whh": np.ascontiguousarray(whh_t * np.float32(G_SCALE)),         # [512, 2048]
        "w8": w8,
        "bias": np.ascontiguousarray(bias_both),
    }
    in_maps = []
    for c in range(N_CORES):
        m = dict(shared)
        m["x"] = np.ascontiguousarray(
            x[c * B_PER_CORE:(c + 1) * B_PER_CORE].reshape(NT, D)
        )
        in_maps.append(m)
    return in_maps


def kernel(x, conv_w, conv_b, w_ih, w_hh, b_ih, b_hh):
    if "nc" not in _cache:
        _cache["nc"] = _build()
    nc = _cache["nc"]

    in_maps = _make_in_maps(dict(
        x=x, conv_w=conv_w, conv_b=conv_b, w_ih=w_ih, w_hh=w_hh,
        b_ih=b_ih, b_hh=b_hh,
    ))

    res = bass_utils.run_bass_kernel_spmd(nc, in_maps, core_ids=list(range(N_CORES)))
    out = np.concatenate(
        [np.ascontiguousarray(r["out"].T).reshape(B_PER_CORE, L, D)
         for r in res.results], axis=0
    )
    return out
